# revision 4
# baseline (speedup 1.0000x reference)
"""Chamfer distance kernel for Trainium2 (8 NeuronCores, SPMD).

Problem: B=16 batches of two 4096-point 3D clouds; cost =
  sum_b 0.5*(mean_n min_m d2[b,n,m] + mean_m min_n d2[b,n,m]).

Sharding: data-parallel over batch; each of the 8 cores handles 2
batches, one pass over each 4096x4096 pair matrix serving BOTH
reduction directions.

Device algorithm (per core, per batch):
  The PE computes d2[n,m] directly in PSUM as a K=15 bf16 matmul: the
  host splits every operand hi/lo into exactly-representable bf16
  pieces (coords 2-level, |p|^2 / |q|^2 3-level), so each partial
  product is exact in fp32 and d2 error is ~1e-5 absolute.
    lhsT rows: xh yh zh xl yl zl 1 1 1 sp1 sp2 sp3 (P cloud)
    rhs  rows: -2Xh -2Yh -2Zh -2Xl -2Yl -2Zl sq1 sq2 sq3 1 1 1 (Q)
  (pairing: cross terms drop only lo*lo ~ 1e-5.)

  Per 128-row n-tile, m in 2 halves of 2048 (PSUM ping-pong, 4 banks
  each):
  - Act drains PSUM once: Y = d2 as f16 (activation copy).
  - DVE dir-2: colA = min(colA, Y)  (f16 2x mode, running column min)
  - DVE dir-1: fold chain min(Y_lo, Y_hi) 4096->256 + reduce -> rmin
    column per tile (f16 2x folds).
  Finale per batch: PE-transpose colA blocks (f16, into bitcast PSUM
  views of the pp tiles) and 3D-view min-reduce across partitions ->
  cmin [128,32]; DMA rmin [128,32] and cmin to HBM; host averages in
  fp64. (Pool's C-axis reduce also works but its Q7 software loop is
  ~ms-scale on real hardware.)

  Engines: DVE ~4.2us/tile (bottleneck), Act ~3.4us, PE ~2.5us, all
  static instructions (no hardware loops, no SWDGE).
"""

import sys

sys.path.insert(0, "/opt/trn_rl_repo")

from contextlib import ExitStack

import numpy as np

import concourse.bass as bass
import concourse.tile as tile
from concourse import bacc, mybir
from concourse.bass_utils import run_bass_kernel_spmd
from concourse.masks import make_identity

B, N, D = 16, 4096, 3
NCORES = 8
BPC = B // NCORES      # batches per core
K = 15                 # contraction rows (hi/lo split, see module doc)
NT = N // 128          # 32 n-tiles per batch
AVG_SCALE = 0.5
F32 = mybir.dt.float32
F16 = mybir.dt.float16
BF16 = mybir.dt.bfloat16
MIN = mybir.AluOpType.min
X = mybir.AxisListType.X

_NC = {}


def _build():
    nc = bacc.Bacc("TRN2", target_bir_lowering=False, debug=False)
    # lhsT / rhs rows for both local batches, batch-major on free axis
    sl = nc.dram_tensor("sl", [K, BPC * N], BF16, kind="ExternalInput").ap()
    rr = nc.dram_tensor("rr", [K, BPC * N], BF16, kind="ExternalInput").ap()
    # out per batch: row-mins [128,NT]; col-mins [128,32] (block-major)
    ro = nc.dram_tensor("ro", [BPC, 128, NT], F32, kind="ExternalOutput").ap()
    co = nc.dram_tensor("co", [BPC, N], F32, kind="ExternalOutput").ap()

    with tile.TileContext(nc) as tc, ExitStack() as ctx:
        sb = ctx.enter_context(tc.tile_pool(name="sb", bufs=1))
        ps = ctx.enter_context(tc.tile_pool(name="ps", bufs=1, space="PSUM"))
        SL = sb.tile([K, BPC * N], BF16, tag="SL")
        RR = sb.tile([K, BPC * N], BF16, tag="RR")
        nc.sync.dma_start(SL[:], sl)
        nc.sync.dma_start(RR[:], rr)

        pp = [ps.tile([128, 2048], F32, name=f"pp{i}", tag=f"pp{i}")
              for i in range(2)]
        NY = 3
        Y = [sb.tile([128, N], F16, name=f"Y{i}", tag=f"Y{i}")
             for i in range(NY)]
        colA = sb.tile([128, N], F16, tag="colA")
        F1 = sb.tile([128, 2048], F16, tag="F1")
        F2 = sb.tile([128, 1024], F16, tag="F2")
        F3 = sb.tile([128, 512], F16, tag="F3")
        F4 = sb.tile([128, 256], F16, tag="F4")
        rmin = sb.tile([128, NT], F32, tag="rmin")
        cmin = sb.tile([128, 32], F32, tag="cmin")
        ident = sb.tile([128, 128], F16, tag="ident")
        make_identity(nc, ident[:])

        for bl in range(BPC):
            b0 = bl * N
            for t in range(NT):
                dst = colA if t == 0 else Y[t % NY]
                for h in range(2):
                    p = pp[(2 * t + h) % 2]
                    for mc in range(4):
                        o = h * 2048 + mc * 512
                        nc.tensor.matmul(
                            p[:, mc * 512:(mc + 1) * 512],
                            SL[:, b0 + t * 128:b0 + (t + 1) * 128],
                            RR[:, b0 + o:b0 + o + 512],
                            start=True, stop=True)
                    # stage d2 as f16 (frees PSUM)
                    nc.scalar.copy(dst[:, h * 2048:(h + 1) * 2048], p[:])
                if t > 0:
                    nc.vector.tensor_tensor(colA[:], dst[:], colA[:], op=MIN)
                # dir-1: fold 4096->256, then reduce -> rmin col t
                nc.vector.tensor_tensor(F1[:], dst[:, 0:2048],
                                        dst[:, 2048:4096], op=MIN)
                nc.vector.tensor_tensor(F2[:], F1[:, 0:1024],
                                        F1[:, 1024:2048], op=MIN)
                nc.vector.tensor_tensor(F3[:], F2[:, 0:512],
                                        F2[:, 512:1024], op=MIN)
                nc.vector.tensor_tensor(F4[:], F3[:, 0:256],
                                        F3[:, 256:512], op=MIN)
                nc.vector.tensor_reduce(rmin[:, t:t + 1], F4[:], op=MIN,
                                        axis=X)
            nc.sync.dma_start(ro[bl], rmin[:])
            # dir-2 finale: transpose colA (f16) into bitcast views of the
            # pp PSUM tiles, then min across partitions via 3D-view reduce
            for g in range(4):
                pv = pp[g % 2][:].bitcast(F16)
                for j in range(8):
                    gb = g * 8 + j
                    nc.tensor.transpose(pv[:, j * 128:(j + 1) * 128],
                                        colA[:, gb * 128:(gb + 1) * 128],
                                        ident[:])
                view = pv[:, 0:1024].rearrange("p (a b) -> p a b", b=128)
                nc.vector.tensor_reduce(cmin[:, g * 8:(g + 1) * 8], view,
                                        op=MIN, axis=X)
            nc.sync.dma_start(co[bl], cmin[:])

    nc.compile()
    return nc


def get_nc(mode=None):
    if "nc" not in _NC:
        _NC["nc"] = _build()
    return _NC["nc"]


def _bf16(x):
    u = np.asarray(x, np.float32).view(np.uint32)
    return ((u + 0x7FFF + ((u >> 16) & 1)) & 0xFFFF0000).view(np.float32)


def _prep_inputs(points1, points2, mode=None):
    """Full inputs -> per-core {"sl": [K,8192] bf16, "rr": ...} maps."""
    import ml_dtypes

    p1 = np.asarray(points1, np.float32)
    p2 = np.asarray(points2, np.float32)
    # hi/lo coordinate split  [B, N, 3]
    p1h = _bf16(p1)
    p1l = _bf16(p1 - p1h)
    p2h = _bf16(p2)
    p2l = _bf16(p2 - p2h)
    # squared norms of the EFFECTIVE (split) coords, 3-level split
    sq1 = ((p1h.astype(np.float64) + p1l) ** 2).sum(-1)   # [B, N]
    sq2 = ((p2h.astype(np.float64) + p2l) ** 2).sum(-1)

    def split3(v):
        a = _bf16(v.astype(np.float32))
        r = v - a.astype(np.float64)
        b = _bf16(r.astype(np.float32))
        r2 = r - b.astype(np.float64)
        c = _bf16(r2.astype(np.float32))
        return a, b, c

    s1a, s1b, s1c = split3(sq1)
    s2a, s2b, s2c = split3(sq2)

    maps = []
    for cid in range(NCORES):
        sl = np.zeros((K, BPC * N), np.float32)
        rr = np.zeros((K, BPC * N), np.float32)
        for bl in range(BPC):
            gb = cid * BPC + bl
            s = slice(bl * N, (bl + 1) * N)
            # k 0-2:  p1h . (-2 p2h)   k 3-5: p1h . (-2 p2l)
            # k 6-8:  p1l . (-2 p2h)   k 9-11: 1 . sq2{a,b,c}
            # k 12-14: sq1{a,b,c} . 1
            sl[0:3, s] = p1h[gb].T
            sl[3:6, s] = p1h[gb].T
            sl[6:9, s] = p1l[gb].T
            sl[9:12, s] = 1.0
            sl[12, s] = s1a[gb]
            sl[13, s] = s1b[gb]
            sl[14, s] = s1c[gb]
            rr[0:3, s] = -2.0 * p2h[gb].T
            rr[3:6, s] = -2.0 * p2l[gb].T
            rr[6:9, s] = -2.0 * p2h[gb].T
            rr[9, s] = s2a[gb]
            rr[10, s] = s2b[gb]
            rr[11, s] = s2c[gb]
            rr[12:15, s] = 1.0
        maps.append({
            "sl": sl.astype(ml_dtypes.bfloat16),
            "rr": rr.astype(ml_dtypes.bfloat16),
        })
    return maps


def _assemble(results):
    total = 0.0
    for cid in range(NCORES):
        r = results[cid]
        ro = r["ro"].astype(np.float64)   # [BPC, 128, NT] rowmins
        co = r["co"].astype(np.float64)   # [BPC, N] colmins (block-major)
        for bl in range(BPC):
            m1 = ro[bl].mean()
            m2 = co[bl].mean()
            total += AVG_SCALE * (m1 + m2)
    return np.asarray(total, dtype=np.float32)


def run(points1, points2, trace=False, tmpdir=None, mode=None):
    nc = get_nc()
    in_maps = _prep_inputs(points1, points2)
    res = run_bass_kernel_spmd(nc, in_maps, list(range(NCORES)),
                               trace=trace, tmpdir=tmpdir)
    return _assemble(res.results), res


def kernel(points1, points2):
    out, _ = run(points1, points2)
    return out


# revision 17
# speedup vs baseline: 1.2643x; 1.2643x over previous
"""Chamfer distance kernel for Trainium2 (8 NeuronCores, SPMD).

Problem: B=16 batches of two 4096-point 3D clouds; cost =
  sum_b 0.5*(mean_n min_m d2[b,n,m] + mean_m min_n d2[b,n,m]).

Sharding: data-parallel over batch; each of the 8 cores handles 2
batches, one pass over each 4096x4096 pair matrix serving BOTH
reduction directions.

Device algorithm (per core, per batch):
  The PE computes d2[n,m] directly in PSUM as a K=13 f16 matmul: the
  host splits every operand hi/lo into exactly-representable f16
  pieces (coords 2-level ~2^-22 residual, |p|^2 / |q|^2 2-level), so
  each partial product is exact in fp32 and d2 error is ~1e-5.
    k 0-2:  p1h . (-2 p2h)    k 3-5: p1h . (-2 p2l)
    k 6-8:  p1l . (-2 p2h)    k 9-10: 1 . sq2{a,b}
    k 11-12: sq1{a,b} . 1     (dropped lo*lo cross terms ~1e-6)

  Per 128-row n-tile, m in 2 halves of 2048 (PSUM ping-pong, 4 banks
  each):
  - Act drains PSUM once: Y = d2 as f16 (activation copy).
  - DVE dir-2: colA = min(colA, Y)  (f16 2x mode, running column min)
  - DVE dir-1: fold chain min(Y_lo, Y_hi) 4096->256 + reduce -> rmin
    column per tile (f16 2x folds).
  Finale per batch: PE-transpose colA blocks (f16, into bitcast PSUM
  views of the pp tiles) and 3D-view min-reduce across partitions ->
  cmin [128,32]; DMA rmin [128,32] and cmin to HBM; host averages in
  fp64. (Pool's C-axis reduce also works but its Q7 software loop is
  ~ms-scale on real hardware.)

  Engines per tile: DVE ~4.3us (bottleneck, 94% busy), Act ~3.4us,
  PE ~1.9us; fully static program (no hardware loops, no SWDGE, no
  drains). TimelineSim: ~330us/core; R-loop-measured HW: ~300-550us
  (vs baseline kernel's ~1.3-1.8ms measured the same way).
"""

import sys

sys.path.insert(0, "/opt/trn_rl_repo")

from contextlib import ExitStack

import numpy as np

import concourse.bass as bass
import concourse.tile as tile
from concourse import bacc, mybir
from concourse.bass_utils import run_bass_kernel_spmd
from concourse.masks import make_identity

B, N, D = 16, 4096, 3
NCORES = 8
BPC = B // NCORES      # batches per core
K = 13                 # contraction rows (hi/lo split, see module doc)
NT = N // 128          # 32 n-tiles per batch
AVG_SCALE = 0.5
F32 = mybir.dt.float32
F16 = mybir.dt.float16
BF16 = mybir.dt.bfloat16
MIN = mybir.AluOpType.min
X = mybir.AxisListType.X

_NC = {}


def _build(repeat=1):
    # repeat>1 wraps the whole program in a hardware loop; used only by
    # timing probes (the shipped kernel stays a fully static program).
    nc = bacc.Bacc("TRN2", target_bir_lowering=False, debug=False)
    # lhsT / rhs rows for both local batches, batch-major on free axis
    # 10 shipped rows per side (p1h x3, p1l x3, ones x2, sq x2); the
    # duplicated hi rows are read twice from DRAM by the loader DMAs
    sl = nc.dram_tensor("sl", [10, BPC * N], F16, kind="ExternalInput").ap()
    rr = nc.dram_tensor("rr", [10, BPC * N], F16, kind="ExternalInput").ap()
    # out per batch: partial sums of row-mins / col-mins over partitions
    ro = nc.dram_tensor("ro", [BPC, 128], F32, kind="ExternalOutput").ap()
    co = nc.dram_tensor("co", [BPC, 128], F32, kind="ExternalOutput").ap()

    with tile.TileContext(nc) as tc, ExitStack() as ctx:
        sb = ctx.enter_context(tc.tile_pool(name="sb", bufs=1))
        ps = ctx.enter_context(tc.tile_pool(name="ps", bufs=1, space="PSUM"))
        SL = sb.tile([K, BPC * N], F16, tag="SL")
        RR = sb.tile([K, BPC * N], F16, tag="RR")
        pp = [ps.tile([128, 2048], F32, name=f"pp{i}", tag=f"pp{i}")
              for i in range(2)]
        NY = 4
        Y = [sb.tile([128, N], F16, name=f"Y{i}", tag=f"Y{i}")
             for i in range(NY)]
        colA = [sb.tile([128, N], F16, name=f"colA{i}", tag=f"colA{i}")
                for i in range(BPC)]
        F1 = sb.tile([128, 2048], F16, tag="F1")
        F2 = sb.tile([128, 1024], F16, tag="F2")
        F3 = sb.tile([128, 512], F16, tag="F3")
        F4 = sb.tile([128, 512], F16, tag="F4")
        rmin = [sb.tile([128, NT], F32, name=f"rmin{i}", tag=f"rmin{i}")
                for i in range(BPC)]
        cmin = sb.tile([128, 32], F32, tag="cmin")
        rsum = sb.tile([128, 1], F32, tag="rsum")
        csum = sb.tile([128, 1], F32, tag="csum")
        ident = sb.tile([128, 128], F16, tag="ident")
        if repeat > 1:
            ctx.enter_context(tc.For_i(0, repeat, 1))
        # shipped layout both sides: 0-2 hi, 3-5 lo, 6-7 ones, 8-9 sq
        # SL rows: 0-2 p1h, 3-5 p1h(dup), 6-8 p1l, 9-10 ones, 11-12 sq1
        nc.sync.dma_start(SL[0:3, :], sl[0:3, :])
        nc.sync.dma_start(SL[3:6, :], sl[0:3, :])
        nc.sync.dma_start(SL[6:9, :], sl[3:6, :])
        nc.sync.dma_start(SL[9:11, :], sl[6:8, :])
        nc.sync.dma_start(SL[11:13, :], sl[8:10, :])
        # RR rows: 0-2 -2p2h, 3-5 -2p2l, 6-8 -2p2h(dup), 9-10 sq2,
        # 11-12 ones
        nc.sync.dma_start(RR[0:3, :], rr[0:3, :])
        nc.sync.dma_start(RR[3:6, :], rr[3:6, :])
        nc.sync.dma_start(RR[6:9, :], rr[0:3, :])
        nc.sync.dma_start(RR[9:11, :], rr[8:10, :])
        nc.sync.dma_start(RR[11:13, :], rr[6:8, :])
        make_identity(nc, ident[:])

        for bl in range(BPC):
            b0 = bl * N
            cA = colA[bl]
            for t in range(NT):
                dst = cA if t == 0 else Y[t % NY]
                for h in range(2):
                    p = pp[(2 * t + h) % 2]
                    for mc in range(4):
                        o = h * 2048 + mc * 512
                        nc.tensor.matmul(
                            p[:, mc * 512:(mc + 1) * 512],
                            SL[:, b0 + t * 128:b0 + (t + 1) * 128],
                            RR[:, b0 + o:b0 + o + 512],
                            start=True, stop=True)
                    # stage d2 as f16 (frees PSUM)
                    nc.scalar.copy(dst[:, h * 2048:(h + 1) * 2048], p[:])
                if t > 0:
                    nc.vector.tensor_tensor(cA[:], dst[:], cA[:], op=MIN)
                # dir-1: fold 4096->256, then reduce -> rmin col t
                nc.vector.tensor_tensor(F1[:], dst[:, 0:2048],
                                        dst[:, 2048:4096], op=MIN)
                nc.vector.tensor_tensor(F2[:], F1[:, 0:1024],
                                        F1[:, 1024:2048], op=MIN)
                nc.vector.tensor_tensor(F3[:], F2[:, 0:512],
                                        F2[:, 512:1024], op=MIN)
                nc.vector.tensor_tensor(F4[:, (t % 2) * 256:
                                           (t % 2) * 256 + 256],
                                        F3[:, 0:256], F3[:, 256:512],
                                        op=MIN)
                if t % 2 == 1:
                    nc.vector.tensor_reduce(
                        rmin[bl][:, t - 1:t + 1],
                        F4[:].rearrange("p (a b) -> p a b", b=256),
                        op=MIN, axis=X)
            nc.vector.tensor_reduce(rsum[:], rmin[bl][:],
                                    op=mybir.AluOpType.add, axis=X)
            nc.sync.dma_start(ro[bl], rsum[:])
        # dir-2 finales (after both batches: batch-0 finale overlaps
        # batch-1 main loop on PE/DVE gaps): transpose colA (f16) into
        # bitcast views of the pp PSUM tiles, then min across partitions
        for bl in range(BPC):
            for g in range(4):
                pv = pp[g % 2][:].bitcast(F16)
                for j in range(8):
                    gb = g * 8 + j
                    nc.tensor.transpose(pv[:, j * 128:(j + 1) * 128],
                                        colA[bl][:, gb * 128:(gb + 1) * 128],
                                        ident[:])
                view = pv[:, 0:1024].rearrange("p (a b) -> p a b", b=128)
                nc.vector.tensor_reduce(cmin[:, g * 8:(g + 1) * 8], view,
                                        op=MIN, axis=X)
            nc.vector.tensor_reduce(csum[:], cmin[:],
                                    op=mybir.AluOpType.add, axis=X)
            nc.sync.dma_start(co[bl], csum[:])

    nc.compile()
    return nc


def get_nc(mode=None):
    if "nc" not in _NC:
        _NC["nc"] = _build()
    return _NC["nc"]


def _f16(x):
    return np.asarray(x, np.float32).astype(np.float16).astype(np.float32)


def _prep_inputs(points1, points2, mode=None):
    """Full inputs -> per-core {"sl": [K,8192] f16, "rr": ...} maps."""
    p1 = np.asarray(points1, np.float32)
    p2 = np.asarray(points2, np.float32)
    # hi/lo coordinate split  [B, N, 3]
    p1h = _f16(p1)
    p1l = _f16(p1 - p1h)
    p2h = _f16(p2)
    p2l = _f16(p2 - p2h)
    # squared norms of the EFFECTIVE (split) coords, 2-level split
    sq1 = ((p1h.astype(np.float64) + p1l) ** 2).sum(-1)   # [B, N]
    sq2 = ((p2h.astype(np.float64) + p2l) ** 2).sum(-1)

    def split2(v):
        a = _f16(v.astype(np.float32))
        b = _f16((v - a.astype(np.float64)).astype(np.float32))
        return a, b

    s1a, s1b = split2(sq1)
    s2a, s2b = split2(sq2)

    maps = []
    for cid in range(NCORES):
        sl = np.zeros((10, BPC * N), np.float32)
        rr = np.zeros((10, BPC * N), np.float32)
        sl[6:8] = 1.0
        rr[6:8] = 1.0
        for bl in range(BPC):
            gb = cid * BPC + bl
            s = slice(bl * N, (bl + 1) * N)
            sl[0:3, s] = p1h[gb].T
            sl[3:6, s] = p1l[gb].T
            sl[8, s] = s1a[gb]
            sl[9, s] = s1b[gb]
            rr[0:3, s] = -2.0 * p2h[gb].T
            rr[3:6, s] = -2.0 * p2l[gb].T
            rr[8, s] = s2a[gb]
            rr[9, s] = s2b[gb]
        maps.append({
            "sl": sl.astype(np.float16),
            "rr": rr.astype(np.float16),
        })
    return maps


def _assemble(results):
    total = 0.0
    for cid in range(NCORES):
        r = results[cid]
        ro = r["ro"].astype(np.float64)   # [BPC, 128] partial rowmin sums
        co = r["co"].astype(np.float64)   # [BPC, 128] partial colmin sums
        for bl in range(BPC):
            m1 = ro[bl].sum() / N
            m2 = co[bl].sum() / N
            total += AVG_SCALE * (m1 + m2)
    return np.asarray(total, dtype=np.float32)


def run(points1, points2, trace=False, tmpdir=None, mode=None):
    nc = get_nc()
    in_maps = _prep_inputs(points1, points2)
    res = run_bass_kernel_spmd(nc, in_maps, list(range(NCORES)),
                               trace=trace, tmpdir=tmpdir)
    return _assemble(res.results), res


def kernel(points1, points2):
    out, _ = run(points1, points2)
    return out


# revision 19
# speedup vs baseline: 1.3552x; 1.0719x over previous
"""Chamfer distance kernel for Trainium2 (8 NeuronCores, SPMD).

Problem: B=16 batches of two 4096-point 3D clouds; cost =
  sum_b 0.5*(mean_n min_m d2[b,n,m] + mean_m min_n d2[b,n,m]).

Sharding: data-parallel over batch; each of the 8 cores handles 2
batches, one pass over each 4096x4096 pair matrix serving BOTH
reduction directions.

Device algorithm (per core, per batch):
  The PE computes d2[n,m] directly in PSUM as a K=13 f16 matmul: the
  host splits every operand hi/lo into exactly-representable f16
  pieces (coords 2-level ~2^-22 residual, |p|^2 / |q|^2 2-level), so
  each partial product is exact in fp32 and d2 error is ~1e-5.
    k 0-2:  p1h . (-2 p2h)    k 3-5: p1h . (-2 p2l)
    k 6-8:  p1l . (-2 p2h)    k 9-10: 1 . sq2{a,b}
    k 11-12: sq1{a,b} . 1     (dropped lo*lo cross terms ~1e-6)

  Per 128-row n-tile, m in 2 halves of 2048 (PSUM ping-pong, 4 banks
  each):
  - Act drains PSUM once: Y = d2 as f16 (activation copy).
  - DVE dir-2: colA = min(colA, Y)  (f16 2x mode, running column min)
  - DVE dir-1: fold chain min(Y_lo, Y_hi) 4096->256 + reduce -> rmin
    column per tile (f16 2x folds).
  Finale per batch: PE-transpose colA blocks (f16, into bitcast PSUM
  views of the pp tiles) and 3D-view min-reduce across partitions ->
  cmin [128,32]; rmin/cmin are ADD-reduced to [128,1] partial sums and
  DMAed out; the host finishes the means in fp64. (Pool's C-axis
  reduce also works but its Q7 software loop is ~ms-scale on real HW.)

  Engines per tile: DVE ~4.3us (bottleneck, 94% busy), Act ~3.4us,
  PE ~1.9us; fully static program (no hardware loops, no SWDGE, no
  drains). TimelineSim: ~330us/core; R-loop-measured HW: ~300-550us
  (vs baseline kernel's ~1.3-1.8ms measured the same way).
"""

import sys

sys.path.insert(0, "/opt/trn_rl_repo")

from contextlib import ExitStack

import numpy as np

import concourse.bass as bass
import concourse.tile as tile
from concourse import bacc, mybir
from concourse.bass_utils import run_bass_kernel_spmd
from concourse.masks import make_identity

B, N, D = 16, 4096, 3
NCORES = 8
BPC = B // NCORES      # batches per core
K = 13                 # contraction rows (hi/lo split, see module doc)
NT = N // 128          # 32 n-tiles per batch
AVG_SCALE = 0.5
F32 = mybir.dt.float32
F16 = mybir.dt.float16
BF16 = mybir.dt.bfloat16
MIN = mybir.AluOpType.min
X = mybir.AxisListType.X

_NC = {}


def _build(repeat=1):
    # repeat>1 wraps the whole program in a hardware loop; used only by
    # timing probes (the shipped kernel stays a fully static program).
    nc = bacc.Bacc("TRN2", target_bir_lowering=False, debug=False)
    # lhsT / rhs rows for both local batches, batch-major on free axis
    # 10 shipped rows per side (p1h x3, p1l x3, ones x2, sq x2); the
    # duplicated hi rows are read twice from DRAM by the loader DMAs
    sl = nc.dram_tensor("sl", [10, BPC * N], F16, kind="ExternalInput").ap()
    rr = nc.dram_tensor("rr", [10, BPC * N], F16, kind="ExternalInput").ap()
    # out per batch: partial sums of row-mins / col-mins over partitions
    ro = nc.dram_tensor("ro", [BPC, 128], F32, kind="ExternalOutput").ap()
    co = nc.dram_tensor("co", [BPC, 128], F32, kind="ExternalOutput").ap()

    with tile.TileContext(nc) as tc, ExitStack() as ctx:
        sb = ctx.enter_context(tc.tile_pool(name="sb", bufs=1))
        ps = ctx.enter_context(tc.tile_pool(name="ps", bufs=1, space="PSUM"))
        SL = sb.tile([K, BPC * N], F16, tag="SL")
        RR = sb.tile([K, BPC * N], F16, tag="RR")
        pp = [ps.tile([128, 2048], F32, name=f"pp{i}", tag=f"pp{i}")
              for i in range(2)]
        NY = 4
        Y = [sb.tile([128, N], F16, name=f"Y{i}", tag=f"Y{i}")
             for i in range(NY)]
        colA = [sb.tile([128, N], F16, name=f"colA{i}", tag=f"colA{i}")
                for i in range(BPC)]
        F1 = sb.tile([128, 4096], F16, tag="F1")
        F2 = sb.tile([128, 2048], F16, tag="F2")
        F3 = sb.tile([128, 1024], F16, tag="F3")
        F4 = sb.tile([128, 512], F16, tag="F4")
        rmin = [sb.tile([128, NT], F32, name=f"rmin{i}", tag=f"rmin{i}")
                for i in range(BPC)]
        cmin = sb.tile([128, 32], F32, tag="cmin")
        rsum = sb.tile([128, 1], F32, tag="rsum")
        csum = sb.tile([128, 1], F32, tag="csum")
        ident = sb.tile([128, 128], F16, tag="ident")
        if repeat > 1:
            ctx.enter_context(tc.For_i(0, repeat, 1))
        # shipped layout both sides: 0-2 hi, 3-5 lo, 6-7 ones, 8-9 sq
        # SL rows: 0-2 p1h, 3-5 p1h(dup), 6-8 p1l, 9-10 ones, 11-12 sq1
        nc.sync.dma_start(SL[0:3, :], sl[0:3, :])
        nc.sync.dma_start(SL[3:6, :], sl[0:3, :])
        nc.sync.dma_start(SL[6:9, :], sl[3:6, :])
        nc.sync.dma_start(SL[9:11, :], sl[6:8, :])
        nc.sync.dma_start(SL[11:13, :], sl[8:10, :])
        # RR rows: 0-2 -2p2h, 3-5 -2p2l, 6-8 -2p2h(dup), 9-10 sq2,
        # 11-12 ones
        nc.sync.dma_start(RR[0:3, :], rr[0:3, :])
        nc.sync.dma_start(RR[3:6, :], rr[3:6, :])
        nc.sync.dma_start(RR[6:9, :], rr[0:3, :])
        nc.sync.dma_start(RR[9:11, :], rr[8:10, :])
        nc.sync.dma_start(RR[11:13, :], rr[6:8, :])
        make_identity(nc, ident[:])

        for bl in range(BPC):
            b0 = bl * N
            cA = colA[bl]
            for t in range(NT):
                dst = cA if t == 0 else Y[t % NY]
                for h in range(2):
                    p = pp[(2 * t + h) % 2]
                    for mc in range(4):
                        o = h * 2048 + mc * 512
                        nc.tensor.matmul(
                            p[:, mc * 512:(mc + 1) * 512],
                            SL[:, b0 + t * 128:b0 + (t + 1) * 128],
                            RR[:, b0 + o:b0 + o + 512],
                            start=True, stop=True)
                    # stage d2 as f16 (frees PSUM)
                    nc.scalar.copy(dst[:, h * 2048:(h + 1) * 2048], p[:])
                if t > 0:
                    nc.vector.tensor_tensor(cA[:], dst[:], cA[:], op=MIN)
                # dir-1: fold into tile-pair slot of F1; every odd tile,
                # run fold levels 2-4 + reduce batched over BOTH tiles
                # via 3D strided views (fewer DVE instruction overheads)
                sl2 = (t % 2) * 2048
                nc.vector.tensor_tensor(F1[:, sl2:sl2 + 2048],
                                        dst[:, 0:2048],
                                        dst[:, 2048:4096], op=MIN)
                if t % 2 == 1:
                    v1 = F1[:].rearrange("p (a b) -> p a b", b=2048)
                    v2o = F2[:].rearrange("p (a b) -> p a b", b=1024)
                    nc.vector.tensor_tensor(v2o, v1[:, :, 0:1024],
                                            v1[:, :, 1024:2048], op=MIN)
                    v2 = F2[:].rearrange("p (a b) -> p a b", b=1024)
                    v3o = F3[:].rearrange("p (a b) -> p a b", b=512)
                    nc.vector.tensor_tensor(v3o, v2[:, :, 0:512],
                                            v2[:, :, 512:1024], op=MIN)
                    v3 = F3[:].rearrange("p (a b) -> p a b", b=512)
                    v4o = F4[:].rearrange("p (a b) -> p a b", b=256)
                    nc.vector.tensor_tensor(v4o, v3[:, :, 0:256],
                                            v3[:, :, 256:512], op=MIN)
                    nc.vector.tensor_reduce(
                        rmin[bl][:, t - 1:t + 1],
                        F4[:].rearrange("p (a b) -> p a b", b=256),
                        op=MIN, axis=X)
            nc.vector.tensor_reduce(rsum[:], rmin[bl][:],
                                    op=mybir.AluOpType.add, axis=X)
            nc.sync.dma_start(ro[bl], rsum[:])
        # dir-2 finales (after both batches: batch-0 finale overlaps
        # batch-1 main loop on PE/DVE gaps): transpose colA (f16) into
        # bitcast views of the pp PSUM tiles, then min across partitions
        for bl in range(BPC):
            for g in range(4):
                pv = pp[g % 2][:].bitcast(F16)
                for j in range(8):
                    gb = g * 8 + j
                    nc.tensor.transpose(pv[:, j * 128:(j + 1) * 128],
                                        colA[bl][:, gb * 128:(gb + 1) * 128],
                                        ident[:])
                view = pv[:, 0:1024].rearrange("p (a b) -> p a b", b=128)
                nc.vector.tensor_reduce(cmin[:, g * 8:(g + 1) * 8], view,
                                        op=MIN, axis=X)
            nc.vector.tensor_reduce(csum[:], cmin[:],
                                    op=mybir.AluOpType.add, axis=X)
            nc.sync.dma_start(co[bl], csum[:])

    nc.compile()
    return nc


def get_nc(mode=None):
    if "nc" not in _NC:
        _NC["nc"] = _build()
    return _NC["nc"]


def _f16(x):
    return np.asarray(x, np.float32).astype(np.float16).astype(np.float32)


def _prep_inputs(points1, points2, mode=None):
    """Full inputs -> per-core {"sl": [K,8192] f16, "rr": ...} maps."""
    p1 = np.asarray(points1, np.float32)
    p2 = np.asarray(points2, np.float32)
    # hi/lo coordinate split  [B, N, 3]
    p1h = _f16(p1)
    p1l = _f16(p1 - p1h)
    p2h = _f16(p2)
    p2l = _f16(p2 - p2h)
    # squared norms of the EFFECTIVE (split) coords, 2-level split
    sq1 = ((p1h.astype(np.float64) + p1l) ** 2).sum(-1)   # [B, N]
    sq2 = ((p2h.astype(np.float64) + p2l) ** 2).sum(-1)

    def split2(v):
        a = _f16(v.astype(np.float32))
        b = _f16((v - a.astype(np.float64)).astype(np.float32))
        return a, b

    s1a, s1b = split2(sq1)
    s2a, s2b = split2(sq2)

    maps = []
    for cid in range(NCORES):
        sl = np.zeros((10, BPC * N), np.float32)
        rr = np.zeros((10, BPC * N), np.float32)
        sl[6:8] = 1.0
        rr[6:8] = 1.0
        for bl in range(BPC):
            gb = cid * BPC + bl
            s = slice(bl * N, (bl + 1) * N)
            sl[0:3, s] = p1h[gb].T
            sl[3:6, s] = p1l[gb].T
            sl[8, s] = s1a[gb]
            sl[9, s] = s1b[gb]
            rr[0:3, s] = -2.0 * p2h[gb].T
            rr[3:6, s] = -2.0 * p2l[gb].T
            rr[8, s] = s2a[gb]
            rr[9, s] = s2b[gb]
        maps.append({
            "sl": sl.astype(np.float16),
            "rr": rr.astype(np.float16),
        })
    return maps


def _assemble(results):
    total = 0.0
    for cid in range(NCORES):
        r = results[cid]
        ro = r["ro"].astype(np.float64)   # [BPC, 128] partial rowmin sums
        co = r["co"].astype(np.float64)   # [BPC, 128] partial colmin sums
        for bl in range(BPC):
            m1 = ro[bl].sum() / N
            m2 = co[bl].sum() / N
            total += AVG_SCALE * (m1 + m2)
    return np.asarray(total, dtype=np.float32)


def run(points1, points2, trace=False, tmpdir=None, mode=None):
    nc = get_nc()
    in_maps = _prep_inputs(points1, points2)
    res = run_bass_kernel_spmd(nc, in_maps, list(range(NCORES)),
                               trace=trace, tmpdir=tmpdir)
    return _assemble(res.results), res


def kernel(points1, points2):
    out, _ = run(points1, points2)
    return out
